# revision 14
# baseline (speedup 1.0000x reference)
"""AttentionWithRoPE on 8 trn2 NeuronCores.

Sharding (tensor-parallel over heads x data-parallel over batch):
  core c -> batch b = c // 4, head group g = c % 4 (heads [4g, 4g+4)).
Each core computes q/k/v projections for its 4 heads (columns
[512g, 512g+512) of Wq/Wk/Wv), causal attention with RoPE, and the
partial o_proj contribution  attn_out_local @ Wo[512g:512g+512, :].
The host gather sums the 4 partials per batch (row-parallel linear).

v4: all matmuls bf16; weights + qT/kT/v SBUF-resident; inputs arrive
host-pre-rearranged so every DMA is a dense [128, N] tile (8-16KB
contiguous per partition, no gather descriptors). Per-512-query-block
pipeline: projections / attention / o_proj of adjacent blocks overlap
on the PE. Softmax denominators accumulate on VectorE in bf16 (pair
adds, 2x mode), partition-reduced by two [1,512] ones-matmuls per
head (the all-ones lhsT column is a slice of the causal mask), then
1/L = exp(-ln L) batched per block on ScalarE (one Ln + one Exp on
[1,2048] keeps activation-table switches to 2 per block) and
partition-broadcast on the idle GpSimd. av is evicted to SBUF right
after its last accumulation so one PSUM bank suffices for it.

PSUM banks: proj(v+qk) 2, scores pair 2, av 1, lsum 1, o_proj 2 = 8.
"""

import os
import sys

for _p in ("/opt/trn_rl_repo", "/root/.axon_site/_ro/trn_rl_repo"):
    if _p not in sys.path:
        sys.path.insert(0, _p)

import numpy as np
import ml_dtypes

import concourse.bass as bass
import concourse.tile as tile
from concourse import bacc, bass_isa, mybir
from concourse.bass_utils import run_bass_kernel_spmd

f32 = mybir.dt.float32
bf16 = mybir.dt.bfloat16
EXP = mybir.ActivationFunctionType.Exp
LN = mybir.ActivationFunctionType.Ln
COPY = mybir.ActivationFunctionType.Copy

B = 2
S = 2048
E = 2048
D = 128
HL = 4          # local heads per core
EL = HL * D     # 512, local projection width
NB = S // 512   # 4 query 512-blocks
EC = E // 128   # 16 contraction chunks
SCALE = float(1.0 / np.sqrt(D))

_CACHE = {}


class _PinnedActBacc(bacc.Bacc):
    """Pin every activation to the natural_log_exp_and_others table set.

    The stock table-load pass picks, per ACTIVATE, the first act_info set
    containing its function: Exp resolves to exp_and_others and Ln to
    natural_log_exp_and_others, so a kernel using both thrashes table
    loads (~2.7us each). All functions used here (Exp, Ln, Copy) live in
    natural_log_exp_and_others, so blank out every other set's function
    list (indices must keep act_info.json order) and one load suffices.
    """

    def insert_act_table_loads(self):
        from concourse.hw_specs import get_activation_tables

        keep = "natural_log_exp_and_others"
        tables = [
            (n, fns if n == keep else set())
            for n, fns in get_activation_tables(self.m.arch).items()
        ]
        bacc._bass_rust.insert_act_table_loads(self, tables)


def _build():
    from contextlib import ExitStack

    nc = _PinnedActBacc("TRN2", target_bir_lowering=False, debug=False, num_devices=8)

    # all pre-rearranged on host: partition dim first, contiguous free dims
    HST = nc.dram_tensor("hsT", [128, 2, NB, EC // 2, 512], bf16, kind="ExternalInput")
    WQ = nc.dram_tensor("wq", [128, EC, EL], bf16, kind="ExternalInput")
    WK = nc.dram_tensor("wk", [128, EC, EL], bf16, kind="ExternalInput")
    WV = nc.dram_tensor("wv", [128, EC, EL], bf16, kind="ExternalInput")
    WO = nc.dram_tensor("wo", [128, HL, E], bf16, kind="ExternalInput")
    COS = nc.dram_tensor("cosT", [D, S], bf16, kind="ExternalInput")
    SIN = nc.dram_tensor("sinTs", [D, S], bf16, kind="ExternalInput")  # sign-folded
    MSK = nc.dram_tensor("masks", [128, 4, 512], bf16, kind="ExternalInput")
    OUT = nc.dram_tensor("out", [S, E], f32, kind="ExternalOutput")

    with tile.TileContext(nc) as tc, nc.allow_low_precision("bf16 compute by design"):
        with ExitStack() as ctx:
            res = ctx.enter_context(tc.tile_pool(name="res", bufs=1))
            wv_sb = res.tile([128, EC, EL], bf16, tag="wv")
            wq_sb = res.tile([128, EC, EL], bf16, tag="wq")
            wk_sb = res.tile([128, EC, EL], bf16, tag="wk")
            wo_sb = res.tile([128, HL, E], bf16, tag="wo")
            cos_sb = res.tile([128, S], bf16, tag="cos")
            sin_sb = res.tile([128, S], bf16, tag="sin")
            masks = res.tile([128, 4, 512], bf16, tag="masks")
            kT = [res.tile([128, S], bf16, tag=f"kT{h}", name=f"kT{h}") for h in range(HL)]
            qT = [res.tile([128, S], bf16, tag=f"qT{h}", name=f"qT{h}") for h in range(HL)]
            v_sb = res.tile([128, NB * 4, EL], bf16, tag="v")
            # masks[:, 0, 511] is 1 for every sk (sq=511 >= sk for all sk<128):
            # a free all-ones lhsT column for the denominator matmuls.
            ones_col = masks[:, 0, 511:512]

            hsp = ctx.enter_context(tc.tile_pool(name="hsp", bufs=3))
            rawp = ctx.enter_context(tc.tile_pool(name="rawp", bufs=2))
            rotp = ctx.enter_context(tc.tile_pool(name="rotp", bufs=2))
            t1p = ctx.enter_context(tc.tile_pool(name="t1p", bufs=2))
            exp_p = ctx.enter_context(tc.tile_pool(name="exp", bufs=3))
            accp = ctx.enter_context(tc.tile_pool(name="accp", bufs=2))
            lrowp = ctx.enter_context(tc.tile_pool(name="lrowp", bufs=2))
            rrp = ctx.enter_context(tc.tile_pool(name="rrp", bufs=2))
            rbp = ctx.enter_context(tc.tile_pool(name="rbp", bufs=2))
            avsp = ctx.enter_context(tc.tile_pool(name="avsp", bufs=8))
            outp = ctx.enter_context(tc.tile_pool(name="outp", bufs=3))
            # PSUM: proj 2 + sc 2 + av 1 + lsum 1 + op 2 = 8 banks
            pjps = ctx.enter_context(tc.tile_pool(name="pjps", bufs=2, space="PSUM"))
            scps = ctx.enter_context(tc.tile_pool(name="scps", bufs=1, space="PSUM"))
            avps = ctx.enter_context(tc.tile_pool(name="avps", bufs=1, space="PSUM"))
            lsps = ctx.enter_context(tc.tile_pool(name="lsps", bufs=1, space="PSUM"))
            opps = ctx.enter_context(tc.tile_pool(name="opps", bufs=2, space="PSUM"))

            def load_halves(j):
                out = []
                for half in range(2):
                    t = hsp.tile([128, EC // 2, 512], bf16, tag="hscol")
                    nc.sync.dma_start(t[:], HST[:, half, j, :, :])
                    out.append(t)
                return out

            # DMA priority order (sync-engine issue is serial, ~1us per
            # dma_start): first v-chain needs wv chunk 0 + hs block 0;
            # wo is not needed until the first o_proj.
            nc.sync.dma_start(wv_sb[:, 0:4, :], WV[:, 0:4, :])
            halves_next = load_halves(0)
            nc.sync.dma_start(wv_sb[:, 4:16, :], WV[:, 4:16, :])
            nc.scalar.dma_start(wq_sb[:], WQ[:])
            nc.gpsimd.dma_start(wk_sb[:], WK[:])
            nc.gpsimd.dma_start(cos_sb[:], COS[:])
            nc.gpsimd.dma_start(sin_sb[:], SIN[:])
            nc.scalar.dma_start(masks[:], MSK[:])
            nc.scalar.dma_start(wo_sb[:], WO[:])

            def rope_evict(dst, ps, j):
                # dst = raw*cosT + rot(raw)*sinT_signed   (bf16 math)
                raw = rawp.tile([128, 512], bf16, tag="raw")
                nc.vector.tensor_copy(raw[:], ps[:])
                rot = rotp.tile([128, 512], bf16, tag="rot")
                nc.sync.dma_start(rot[0:64, :], raw[64:128, :])
                nc.sync.dma_start(rot[64:128, :], raw[0:64, :])
                t1 = t1p.tile([128, 512], bf16, tag="t1")
                cs = slice(512 * j, 512 * (j + 1))
                nc.vector.tensor_mul(t1[:], raw[:], cos_sb[:, cs])
                nc.vector.tensor_mul(rot[:], rot[:], sin_sb[:, cs])
                nc.vector.tensor_add(dst, t1[:], rot[:])

            for j in range(NB):
                sj = slice(512 * j, 512 * (j + 1))
                halves = halves_next
                if j + 1 < NB:
                    halves_next = load_halves(j + 1)

                # ---- v projection: 4 sequential 16-matmul chains ----
                for i in range(4):
                    vp = pjps.tile([128, EL], f32, tag="pj")
                    for e in range(EC):
                        nc.tensor.matmul(
                            vp[:],
                            halves[e // 8][:, e % 8, i * 128:(i + 1) * 128],
                            wv_sb[:, e, :],
                            start=(e == 0),
                            stop=(e == EC - 1),
                        )
                    nc.vector.tensor_copy(v_sb[:, j * 4 + i, :], vp[:])

                av_sb = []
                for h in range(HL):
                    # ---- q & k projections with fused RoPE eviction ----
                    hs_ = slice(h * 128, (h + 1) * 128)
                    ps = pjps.tile([128, 512], f32, tag="pj")
                    for e in range(EC):
                        nc.tensor.matmul(
                            ps[:],
                            wq_sb[:, e, hs_],
                            halves[e // 8][:, e % 8, :],
                            start=(e == 0),
                            stop=(e == EC - 1),
                        )
                    rope_evict(qT[h][:, sj], ps, j)

                    ps = pjps.tile([128, 512], f32, tag="pj")
                    for e in range(EC):
                        nc.tensor.matmul(
                            ps[:],
                            wk_sb[:, e, hs_],
                            halves[e // 8][:, e % 8, :],
                            start=(e == 0),
                            stop=(e == EC - 1),
                        )
                    rope_evict(kT[h][:, sj], ps, j)

                    # ---- attention for (j, h) ----
                    npair = 2 * j + 2
                    nkb = 4 * j + 4
                    av = avps.tile([128, 512], f32, tag="av")
                    acc = accp.tile([128, 2, 512], bf16, tag="acc")
                    for p in range(npair):
                        # diagonal pairs: off[kk] = first causally-valid
                        # query column (128*m); columns below it are fully
                        # masked, [off, off+128) is the partial strip.
                        diag = p >= 2 * j
                        offs = [0, 0]
                        if diag:
                            m0 = 2 * p - 4 * j
                            offs = [128 * m0, 128 * (m0 + 1)]
                        sc = scps.tile([128, 2, 512], f32, tag="sc")
                        for kk in range(2):
                            kb = 2 * p + kk
                            o = offs[kk] if offs[kk] >= 256 else 0
                            nc.tensor.matmul(
                                sc[:, kk, o:],
                                kT[h][:, kb * 128:(kb + 1) * 128],
                                qT[h][:, 512 * j + o:512 * (j + 1)],
                                start=True,
                                stop=True,
                            )
                        ex = exp_p.tile([128, 2, 512], bf16, tag="ex")
                        if offs[0] >= 256:
                            # split exp around the unwritten PSUM region
                            for kk in range(2):
                                o = offs[kk]
                                nc.vector.memzero(ex[:, kk, :o])
                                nc.scalar.activation(
                                    ex[:, kk, o:], sc[:, kk, o:], EXP,
                                    scale=SCALE,
                                )
                        else:
                            nc.scalar.activation(ex[:], sc[:], EXP, scale=SCALE)
                            if diag and offs[1] > 0:
                                nc.vector.memzero(ex[:, 1, :offs[1]])
                        if diag:  # mask only the partial 128-wide strips
                            for kk in range(2):
                                m = 2 * p - 4 * j + kk
                                o = offs[kk]
                                nc.vector.tensor_mul(
                                    ex[:, kk, o:o + 128],
                                    ex[:, kk, o:o + 128],
                                    masks[:, m, o:o + 128],
                                )
                        if p == 0:
                            nc.vector.tensor_copy(acc[:], ex[:])
                        else:
                            nc.vector.tensor_add(acc[:], acc[:], ex[:])
                        for kk in range(2):
                            kb = 2 * p + kk
                            o = offs[kk] if offs[kk] >= 256 else 0
                            nc.tensor.matmul(
                                av[:, o:],
                                v_sb[:, kb, hs_],
                                ex[:, kk, o:],
                                start=(kb == 0),
                                stop=(kb == nkb - 1),
                            )
                    # partition-reduce the bf16 denominator accumulator,
                    # then 1/L = exp(-ln L): Ln reads PSUM directly (fused
                    # eviction), Exp(scale=-1) inverts, GpSimd broadcasts.
                    lsum = lsps.tile([1, 512], f32, tag="lsum")
                    for kk in range(2):
                        nc.tensor.matmul(
                            lsum[:], ones_col, acc[:, kk, :],
                            start=(kk == 0), stop=(kk == 1),
                        )
                    lnr = lrowp.tile([1, 512], f32, tag="lnr")
                    nc.scalar.activation(lnr[:], lsum[:], LN)
                    rrow = rrp.tile([1, 512], f32, tag="rrow")
                    nc.scalar.activation(rrow[:], lnr[:], EXP, scale=-1.0)
                    rbc = rbp.tile([128, 512], f32, tag="rbc")
                    nc.gpsimd.partition_broadcast(rbc[:], rrow[:])
                    avs = avsp.tile([128, 512], bf16, tag="avsb")
                    nc.vector.tensor_copy(avs[:], av[:])
                    nc.vector.tensor_mul(avs[:], avs[:], rbc[:])
                    av_sb.append(avs)

                # ---- o_proj partial for query rows of block j ----
                for i in range(4):
                    rows = slice(512 * j + 128 * i, 512 * j + 128 * (i + 1))
                    for n in range(4):
                        op = opps.tile([128, 512], f32, tag="op")
                        for h in range(HL):
                            nc.tensor.matmul(
                                op[:],
                                av_sb[h][:, i * 128:(i + 1) * 128],
                                wo_sb[:, h, n * 512:(n + 1) * 512],
                                start=(h == 0),
                                stop=(h == HL - 1),
                            )
                        ot = outp.tile([128, 512], f32, tag="out")
                        if n % 2 == 0:
                            nc.vector.tensor_copy(ot[:], op[:])
                        else:
                            nc.scalar.activation(ot[:], op[:], COPY)
                        nc.sync.dma_start(OUT[rows, n * 512:(n + 1) * 512], ot[:])

    nc.compile()
    return nc


def _get_nc():
    if "nc" not in _CACHE:
        _CACHE["nc"] = _build()
    return _CACHE["nc"]


def _make_masks():
    sk = np.arange(128)[:, None]
    sq = np.arange(512)[None, :]
    m = np.stack([(sq >= sk + 128 * mm) for mm in range(4)], axis=1)
    return m.astype(ml_dtypes.bfloat16)


def kernel(hidden_states, cos, sin, Wq, Wk, Wv, Wo):
    bf = ml_dtypes.bfloat16
    hidden_states = np.asarray(hidden_states, dtype=np.float32)
    cos = np.asarray(cos, dtype=np.float32)
    sin = np.asarray(sin, dtype=np.float32)
    Wq_b = np.asarray(Wq, dtype=np.float32).astype(bf)
    Wk_b = np.asarray(Wk, dtype=np.float32).astype(bf)
    Wv_b = np.asarray(Wv, dtype=np.float32).astype(bf)
    Wo_b = np.asarray(Wo, dtype=np.float32).astype(bf)

    nc = _get_nc()
    masks = _make_masks()

    def arrange_w(wcols):  # [E, 512] -> [128, 16, 512], row e = 128c + p
        return np.ascontiguousarray(wcols.reshape(EC, 128, EL).transpose(1, 0, 2))

    def arrange_wo(wrows):  # [512, E] -> [128, 4, E], row = 128h + d
        return np.ascontiguousarray(wrows.reshape(HL, 128, E).transpose(1, 0, 2))

    def arrange_hst(hs_b):  # [S, E] -> hsT [p, half, j, c, s]
        t = hs_b.T.astype(bf)  # [E, S]
        t = t.reshape(2, 8, 128, NB, 512)  # [half, c, p, j, s]
        return np.ascontiguousarray(t.transpose(2, 0, 3, 1, 4))

    in_maps = []
    hsT = [arrange_hst(hidden_states[b]) for b in range(B)]
    cosT = [np.ascontiguousarray(cos[b].T.astype(bf)) for b in range(B)]
    sinTs = []
    for b in range(B):
        s = np.ascontiguousarray(sin[b].T)
        s[:64] *= -1.0
        sinTs.append(s.astype(bf))
    for c in range(8):
        b, g = c // 4, c % 4
        cols = slice(512 * g, 512 * (g + 1))
        in_maps.append({
            "hsT": hsT[b],
            "wq": arrange_w(Wq_b[:, cols]),
            "wk": arrange_w(Wk_b[:, cols]),
            "wv": arrange_w(Wv_b[:, cols]),
            "wo": arrange_wo(Wo_b[cols, :]),
            "cosT": cosT[b],
            "sinTs": sinTs[b],
            "masks": masks,
        })

    res = run_bass_kernel_spmd(
        nc, in_maps, core_ids=list(range(8)),
        tmpdir=os.environ.get("BASS_KERNEL_TMPDIR"),
    )
    globals()["LAST_RESULTS"] = res
    globals()["LAST_EXEC_NS"] = res.exec_time_ns
    out = np.empty((B, S, E), dtype=np.float32)
    for b in range(B):
        acc = res.results[4 * b]["out"].astype(np.float32)
        for g in range(1, 4):
            acc = acc + res.results[4 * b + g]["out"]
        out[b] = acc
    return out
